# revision 30
# baseline (speedup 1.0000x reference)
"""DigitCaps (CapsNet dynamic routing) Trainium2 Bass kernel.

Full computation per batch element b:
    u_hat[r,c,o] = sum_i u[r,i] * W[r,c,i,o]            (einsum)
    b_log = 0; for 3 iters: coef = softmax_c(b_log); s = sum_r coef*u_hat
                v = squash(s); b_log += sum_o u_hat*v
Output: v from last iteration.  Identity used: b_log(t) = u_hat . Vcum(t)
where Vcum = sum of previous v's, so logits are recomputed from Vcum
each iteration instead of accumulated.

Sharding: data-parallel over batch, 512 -> 8 cores x 64.

Key cost-model-driven choices vs the naive version:
  - All operand layouts (u^T spread, masked uTz variants, W spread) are
    packed on the HOST and DMAed as single contiguous bf16 blocks: no
    strided gather DMAs, no on-device transposes or masking.
  - The z/s reductions run as halving ADD-trees in fp16 (TensorTensor,
    DVE 2x mode) instead of TensorReduce (which has no fast modes).
  - s-product keeps 2x mode via coef2 (coefficients duplicated in o-pairs
    so the broadcast AP stays packed in the last dim).
  - V broadcast and the rq-group combine use partition-offset DVE adds,
    not PE/PSUM, so PSUM belongs entirely to the einsum and the next
    batch-tile's einsum overlaps the tail of the current routing.
  - Elementwise work is split DVE (2x) / GPSIMD by span to balance
    engine occupancy; einsum PSUM evictions rotate DVE/ACT/GPSIMD.
"""

import sys

sys.path.insert(0, "/opt/trn_rl_repo")

import functools
from contextlib import ExitStack

import numpy as np

NCORES = 8
B = 64          # batch per core
BT = 32         # batch tile
R = 1152
C = 10
I = 8
O = 16
CO = C * O      # 160
NK = 72         # r-chunks of 16
RQ_K = 18       # k's per r-quartile
RL = 288        # r_loc per quartile (per partition)
RSP = 16        # rl span for routing passes
NSP = RL // RSP  # 18
POOL_SPANS = (0, 1, 2, 3, 4)  # spans assigned to gpsimd (products+trees);
                        # first spans: the einsum produces their u_hat rows
                        # earliest, so the slow engine starts at pass begin
Z1_POOL = tuple(range(10))    # iter-1 z-pass overlaps the einsum: gpsimd
                              # is idle there and takes twice the spans


def _wslice(w):
    return slice(w * 32, (w + 1) * 32)


def build_bass(phase: str = "full"):
    import concourse.bass as bass
    import concourse.tile as tile
    from concourse import bacc, mybir

    f32 = mybir.dt.float32
    bf16 = mybir.dt.bfloat16
    f16 = mybir.dt.float16
    AX = mybir.AxisListType
    OP = mybir.AluOpType
    AF = mybir.ActivationFunctionType

    nc = bacc.Bacc(
        "TRN2",
        target_bir_lowering=False,
        debug=False,
        enable_asserts=False,
        num_devices=NCORES,
    )
    # Host-packed operands (bf16, contiguous):
    #   ut   [128, NK, B]    u^T spread: partition p = 8*m + i, r = 16k + m
    #   utz  [4, 128, NK, B] ut with only rows m%4 == j kept
    #   wsb  [128, NK, C, O] W spread: same partition map
    ut_d = nc.dram_tensor("ut", [128, NK, B], bf16, kind="ExternalInput").ap()
    utz_d = nc.dram_tensor("utz", [4, 128, NK, B], bf16, kind="ExternalInput").ap()
    w_d = nc.dram_tensor("wsb", [128, NK, C, O], bf16, kind="ExternalInput").ap()
    v_d = nc.dram_tensor("v", [B, C, O], f32, kind="ExternalOutput").ap()

    with tile.TileContext(nc) as tc, ExitStack() as ctx:
        # ---------------- persistent pools ----------------
        persist = ctx.enter_context(tc.tile_pool(name="persist", bufs=1))
        uTz0 = persist.tile([128, NK, B], bf16)
        uTz1 = persist.tile([128, NK, B], bf16)
        uTz2 = persist.tile([128, NK, B], bf16)
        uTz3 = persist.tile([128, NK, B], bf16)
        uTz = [uTz0, uTz1, uTz2, uTz3]
        W_sb = persist.tile([128, NK, C, O], bf16)

        rt = ctx.enter_context(tc.tile_pool(name="rt", bufs=1))
        logitsH = rt.tile([128, RL, C], f16)
        E = rt.tile([128, RL, C], bf16)
        den = rt.tile([128, RL], f32)
        coef2 = rt.tile([128, RL, C, 2], bf16)
        V_rep = rt.tile([128, C, O], bf16)
        s_pairD = rt.tile([128, 2, CO], f16)
        s_pairP = rt.tile([128, 2, CO], f16)
        s_stage = rt.tile([32, 3, 2, CO], f16)
        cmb = rt.tile([32, 2, CO], f16)
        v0 = rt.tile([64, C, O], f32)
        Vcb0 = rt.tile([32, C, O], f32)
        Vcb1 = rt.tile([32, C, O], f32)
        Vcb = [Vcb0, Vcb1]
        sm = ctx.enter_context(tc.tile_pool(name="sm", bufs=1))

        def ecopy(which, out_ap, in_ap):
            # PSUM evictions go to DVE and (mostly) ACT; gpsimd is reserved
            # for its routing spans which must start as early as possible.
            if which % 3 == 0:
                nc.vector.tensor_copy(out_ap, in_ap)
            else:
                nc.scalar.copy(out_ap, in_ap)

        def squash(p, s_ap, out_ap, pool):
            # out = |s| / (1 + |s|^2) * s   per (partition, c)
            sq = pool.tile([p, C, O], f32, tag="sqt")
            nc.vector.tensor_mul(sq[:], s_ap, s_ap)
            ssum = pool.tile([p, C], f32, tag="sst")
            nc.vector.tensor_reduce(ssum[:], sq[:], axis=AX.X, op=OP.add)
            norm = pool.tile([p, C], f32, tag="snt")
            nc.scalar.sqrt(norm[:], ssum[:])
            onep = pool.tile([p, C], f32, tag="sot")
            nc.scalar.add(onep[:], ssum[:], 1.0)
            rec = pool.tile([p, C], f32, tag="srt")
            nc.vector.reciprocal(rec[:], onep[:])
            fac = pool.tile([p, C], f32, tag="sft")
            nc.vector.tensor_mul(fac[:], norm[:], rec[:])
            nc.vector.tensor_mul(
                out_ap,
                s_ap,
                fac[:].unsqueeze(2).broadcast_to((p, C, O)),
            )

        def iter0(s0ps):
            s_all = rt.tile([64, C, O], f32)
            nc.scalar.mul(
                s_all[:], s0ps[:].rearrange("p (c o) -> p c o", c=C), 0.1
            )
            squash(64, s_all[:], v0[:], sm)
            nc.vector.tensor_copy(Vcb[0][:], v0[0:32, :, :])
            # cross-partition move must go through DMA (hardware requires
            # compute-op operands to share the same start partition)
            nc.sync.dma_start(out=Vcb[1][:], in_=v0[32:64, :, :])

        # ---------------- prep phase ----------------
        with ExitStack() as prep:
            pp = prep.enter_context(tc.tile_pool(name="prep", bufs=1))
            s0_pool = prep.enter_context(
                tc.tile_pool(name="s0psp", bufs=1, space="PSUM")
            )
            s0ps = s0_pool.tile([64, CO], f32)

            uT_full = pp.tile([128, NK, B], bf16)
            nc.sync.dma_start(out=uT_full[:], in_=ut_d)
            nc.scalar.dma_start(out=W_sb[:], in_=w_d)
            for j in range(4):
                deng = nc.sync if j % 2 == 0 else nc.scalar
                deng.dma_start(out=uTz[j][:], in_=utz_d[j])

            # iter-0 s matmul chain: s0 = sum_k uT_full[:,k,:].T @ W_sb[:,k]
            for k in range(NK):
                nc.tensor.matmul(
                    s0ps[:],
                    uT_full[:, k, :],
                    W_sb[:, k, :, :],
                    start=(k == 0),
                    stop=(k == NK - 1),
                )
            if phase != "prep":
                iter0(s0ps)

        # ---------------- main pools ----------------
        big = ctx.enter_context(tc.tile_pool(name="big", bufs=1))
        scratch = ctx.enter_context(tc.tile_pool(name="scratch", bufs=1))
        u_hat = big.tile([128, RL, C, O], bf16)      # 90 KB/part

        # ---------------- einsum: u_hat per batch tile ----------------
        def einsum_tile(bt, mm_psum):
            # Each MM isolates one r via the zero-masked uTz rows; the 4
            # row-groups (w) and 4 col-groups (rq) tile the PE array.
            for kl in range(RQ_K):
                for j in range(4):
                    pe_ps = mm_psum.tile([128, 4, 512], f32, tag="pe")
                    for rq in range(4):
                        k = rq * RQ_K + kl
                        for w in range(4):
                            nc.tensor.matmul(
                                pe_ps[rq * 32 : (rq + 1) * 32, w, 0:CO],
                                uTz[j][_wslice(w), k, bt * BT : (bt + 1) * BT],
                                W_sb[_wslice(w), k, :, :],
                                start=True,
                                stop=True,
                                tile_position=(w * 32, rq * 32),
                            )
                    rs0 = 16 * kl + j * 4
                    dst = u_hat[:, rs0 : rs0 + 4, :, :].rearrange(
                        "p r c o -> p r (c o)"
                    )
                    ecopy((kl * 4 + j) % 3, dst, pe_ps[:, :, 0:CO])

        # ---------------- routing iteration ----------------
        def vrep_update(bt):
            # V_rep[rq*32+b] = Vcb[bt][b] for all 4 rq groups.  Cast on DVE
            # (same partitions), then three parallel SBUF->SBUF DMAs
            # (compute ops may not mix start partitions on hardware).
            nc.vector.tensor_copy(V_rep[0:32, :, :], Vcb[bt][:])
            nc.sync.dma_start(out=V_rep[32:64, :, :], in_=V_rep[0:32, :, :])
            nc.scalar.dma_start(out=V_rep[64:96, :, :], in_=V_rep[0:32, :, :])
            nc.sync.dma_start(out=V_rep[96:128, :, :], in_=V_rep[0:32, :, :])

        # On gpsimd, TensorTensor mult/add carry a 0.42 software-efficiency
        # penalty but TensorScalarPtr (scalar_tensor_tensor) runs at 0.6 —
        # express gpsimd muls/adds as (in0*1.0) op in1.  DVE keeps plain
        # tensor_tensor (its 2x mode doesn't apply to TensorScalarPtr).
        def e_mul(pool_sp, out, a, b):
            if pool_sp:
                nc.gpsimd.scalar_tensor_tensor(out, a, 1.0, b, OP.mult, OP.mult)
            else:
                nc.vector.tensor_mul(out, a, b)

        def e_add(pool_sp, out, a, b):
            if pool_sp:
                nc.gpsimd.scalar_tensor_tensor(out, a, 1.0, b, OP.mult, OP.add)
            else:
                nc.vector.tensor_add(out, a, b)

        def routing_iter(bt, t):
            vrep_update(bt)
            # During iteration 1 the z-pass overlaps the einsum (gpsimd is
            # otherwise idle there), so gpsimd takes twice the spans.
            z_pool = Z1_POOL if t == 1 else POOL_SPANS
            z_order = list(z_pool) + [
                sp for sp in range(NSP) if sp not in z_pool
            ]
            # ---- z-pass: logits[p, rl, c] = sum_o u_hat * V_rep ----
            for sp in z_order:
                pool_sp = sp in z_pool
                rl0 = sp * RSP
                pr = scratch.tile(
                    [128, RSP, C, O], f16, tag="prp" if pool_sp else "prd"
                )
                e_mul(
                    pool_sp,
                    pr[:],
                    u_hat[:, rl0 : rl0 + RSP, :, :],
                    V_rep[:].unsqueeze(1).broadcast_to((128, RSP, C, O)),
                )
                w = O
                while w > 2:
                    h = w // 2
                    e_add(
                        pool_sp,
                        pr[:, :, :, 0:h], pr[:, :, :, 0:h], pr[:, :, :, h:w]
                    )
                    w = h
                e_add(
                    pool_sp,
                    logitsH[:, rl0 : rl0 + RSP, :],
                    pr[:, :, :, 0],
                    pr[:, :, :, 1],
                )
            # ---- softmax over c (no max-shift: |z| << fp range) ----
            nc.scalar.activation(
                E[:].rearrange("p r c -> p (r c)"),
                logitsH[:].rearrange("p r c -> p (r c)"),
                AF.Exp,
            )
            nc.vector.tensor_reduce(den[:], E[:], axis=AX.X, op=OP.add)
            nc.vector.reciprocal(den[:], den[:])
            # coef2[p, rl, c, 0:2] = E * den^-1 duplicated in o-pairs;
            # gpsimd handles the rows feeding its own s-spans.
            np_ = RSP * len(POOL_SPANS)
            nc.gpsimd.scalar_tensor_tensor(
                coef2[:, 0:np_, :, :],
                E[:, 0:np_, :].unsqueeze(3).broadcast_to((128, np_, C, 2)),
                1.0,
                den[:, 0:np_].unsqueeze(2).unsqueeze(3).broadcast_to(
                    (128, np_, C, 2)
                ),
                OP.mult,
                OP.mult,
            )
            nc.vector.tensor_mul(
                coef2[:, np_:RL, :, :],
                E[:, np_:RL, :].unsqueeze(3).broadcast_to((128, RL - np_, C, 2)),
                den[:, np_:RL].unsqueeze(2).unsqueeze(3).broadcast_to(
                    (128, RL - np_, C, 2)
                ),
            )
            # ---- s-pass: s_pair*[p, 0:2, co] = sum_rl coef * u_hat ----
            s_order = list(POOL_SPANS) + [
                sp for sp in range(NSP) if sp not in POOL_SPANS
            ]
            firstD = True
            firstP = True
            for sp in s_order:
                pool_sp = sp in POOL_SPANS
                rl0 = sp * RSP
                pr2 = scratch.tile(
                    [128, RSP, C, O], f16, tag="prp" if pool_sp else "prd"
                )
                e_mul(
                    pool_sp,
                    pr2[:].rearrange("p r c (e two) -> p (r c) e two", two=2),
                    u_hat[:, rl0 : rl0 + RSP, :, :].rearrange(
                        "p r c (e two) -> p (r c) e two", two=2
                    ),
                    coef2[:, rl0 : rl0 + RSP, :, :]
                    .rearrange("p r c two -> p (r c) two")
                    .unsqueeze(2)
                    .broadcast_to((128, RSP * C, O // 2, 2)),
                )
                w = RSP
                prv = pr2[:].rearrange("p r c o -> p r (c o)")
                while w > 3:
                    h = w // 2
                    e_add(pool_sp, prv[:, 0:h, :], prv[:, 0:h, :], prv[:, h:w, :])
                    w = h
                if w == 3:
                    e_add(pool_sp, prv[:, 0:1, :], prv[:, 0:1, :], prv[:, 2:3, :])
                s_pair = s_pairP if pool_sp else s_pairD
                if (firstP if pool_sp else firstD):
                    if pool_sp:
                        nc.gpsimd.tensor_copy(s_pair[:], prv[:, 0:2, :])
                        firstP = False
                    else:
                        nc.vector.tensor_copy(s_pair[:], prv[:, 0:2, :])
                        firstD = False
                else:
                    e_add(pool_sp, s_pair[:], s_pair[:], prv[:, 0:2, :])
            # ---- combine 4 rq partition groups + o-pairs -> s_bt [32,C,O]
            nc.vector.tensor_add(
                s_pairD[:], s_pairD[:], s_pairP[:]
            )
            # bring the 3 upper rq quarters down to partitions 0:32 via DMA
            for q in range(3):
                deng = (nc.sync, nc.scalar, nc.sync)[q]
                deng.dma_start(
                    out=s_stage[:, q, :, :],
                    in_=s_pairD[32 * (q + 1) : 32 * (q + 2), :, :],
                )
            nc.vector.tensor_add(cmb[:], s_pairD[0:32, :, :], s_stage[:, 0, :, :])
            nc.vector.tensor_add(cmb[:], cmb[:], s_stage[:, 1, :, :])
            nc.vector.tensor_add(cmb[:], cmb[:], s_stage[:, 2, :, :])
            s_bt = sm.tile([32, C, O], f32, tag="sbt")
            nc.vector.tensor_add(
                s_bt[:].rearrange("p c o -> p (c o)"), cmb[:, 0, :], cmb[:, 1, :]
            )

            v_t = sm.tile([32, C, O], f32, tag="vt")
            squash(32, s_bt[:], v_t[:], sm)
            if t == 1:
                nc.vector.tensor_add(Vcb[bt][:], Vcb[bt][:], v_t[:])
            else:
                nc.sync.dma_start(
                    out=v_d[bt * BT : (bt + 1) * BT, :, :], in_=v_t[:]
                )

        if phase == "prep":
            pass
        elif phase == "einsum":
            with ExitStack() as es:
                mm_psum = es.enter_context(
                    tc.tile_pool(name="mmps0", bufs=2, space="PSUM")
                )
                einsum_tile(0, mm_psum)
        elif phase == "iter0":
            with ExitStack() as es:
                mm_psum = es.enter_context(
                    tc.tile_pool(name="mmps0", bufs=2, space="PSUM")
                )
                einsum_tile(0, mm_psum)
                nc.sync.dma_start(out=v_d[0:BT, :, :], in_=v0[0:32, :, :])
        else:
            for bt in range(2):
                with ExitStack() as es:
                    mm_psum = es.enter_context(
                        tc.tile_pool(name=f"mmps{bt}", bufs=2, space="PSUM")
                    )
                    einsum_tile(bt, mm_psum)
                for t in (1, 2):
                    routing_iter(bt, t)

    nc.compile()
    return nc


@functools.cache
def _get_nc():
    return build_bass()


@functools.cache
def _pack_cache():
    return {}


def _pack_inputs(u: np.ndarray, W: np.ndarray):
    import ml_dtypes

    bf = ml_dtypes.bfloat16
    # u: [B, R, I] per core slice -> ut [128=(8m+i), NK, B]
    # W: [R, C, I, O] -> wsb [128=(8m+i), NK, C, O]
    W5 = W.reshape(NK, 16, C, I, O)
    wsb = np.ascontiguousarray(
        W5.transpose(1, 3, 0, 2, 4).reshape(128, NK, C, O)
    ).astype(bf)
    m = (np.arange(128) // I)
    masks = [(m % 4 == j).astype(np.float32)[:, None, None] for j in range(4)]

    def pack_u(u_core):
        u5 = u_core.reshape(B, NK, 16, I)
        ut = np.ascontiguousarray(
            u5.transpose(2, 3, 1, 0).reshape(128, NK, B)
        )
        utz = np.stack([ut * msk for msk in masks], axis=0)
        return ut.astype(bf), utz.astype(bf)

    return wsb, pack_u


def kernel(u: np.ndarray, W: np.ndarray) -> np.ndarray:
    from concourse import bass_utils

    nc = _get_nc()
    W4 = np.ascontiguousarray(W.reshape(R, C, I, O)).astype(np.float32)
    wsb, pack_u = _pack_inputs(u, W4)
    in_maps = []
    for i in range(NCORES):
        ut, utz = pack_u(
            np.ascontiguousarray(u[i * B : (i + 1) * B]).astype(np.float32)
        )
        in_maps.append({"ut": ut, "utz": utz, "wsb": wsb})
    res = bass_utils.run_bass_kernel_spmd(
        nc, in_maps, core_ids=list(range(NCORES))
    )
    return np.concatenate([r["v"] for r in res.results], axis=0)
